# revision 2
# baseline (speedup 1.0000x reference)
"""Multi-head causal attention (B=4, S=2048, D=1024, H=16) on 8 TRN2 NeuronCores.

Sharding: data-parallel over batch (4) x tensor-parallel over heads (2 groups
of 8 heads). Core c handles batch c//2, head group c%2. Each core:
  phase 1: K^T and V projections (fp32r matmuls),
  phase 2: per head-pair, per q-block: Q projection, scores^T = K^T.T @ Q^T
           (two heads row-packed in the 128-wide PE array), exp on ACT,
           causal-mask multiply on DVE, PV accumulation with a ones column
           appended to V so the softmax denominator falls out of the same
           matmul, then normalize via reciprocal + rank-1 broadcast matmul,
  phase 3: row-parallel output projection producing a partial [S, D] result.
Host adds the two partials per batch. All matmuls run as float32r (full-rate
fp32 on the PE). exp is computed without max-subtraction: scores are ~N(0,1)
(|s| < ~7 for these inputs), far inside fp32 exp range, and softmax is
shift-invariant so the result matches the reference.
"""
import numpy as np
import concourse.bass as bass
import concourse.mybir as mybir
import concourse.tile as tile
from concourse import bacc
from concourse.bass_utils import run_bass_kernel_spmd

B, S, D, H, DK = 4, 2048, 1024, 16, 64
EH = 512          # head columns per core (8 heads x 64)
QB = 512          # q-block size
KC = 128          # k-chunk size
NQ = S // QB
NKC = S // KC
NDC = D // 128
VW = DK + 1       # V columns per head incl. ones column

f32 = mybir.dt.float32
f32r = mybir.dt.float32r

_cache = {}
LAST_NC = None
LAST_IN_MAPS = None


def _classify(mask2d):
    """Per (q-block, k-chunk): keep / skip / partial (with dedup'd mask tiles).

    mask2d is [q, k] boolean; device mask tiles are [k, q] float (1=keep).
    """
    kept, partial, patterns, pat_idx = [], {}, [], {}
    for j in range(NQ):
        ks = []
        for c in range(NKC):
            sub = mask2d[j * QB:(j + 1) * QB, c * KC:(c + 1) * KC]
            if not sub.any():
                continue
            ks.append(c)
            if not sub.all():
                t = np.ascontiguousarray(sub.T).astype(np.float32)
                key = t.tobytes()
                if key not in pat_idx:
                    pat_idx[key] = len(patterns)
                    patterns.append(t)
                partial[(j, c)] = pat_idx[key]
        assert ks, f"q-block {j} fully masked; unsupported"
        kept.append(ks)
    if not patterns:
        patterns.append(np.ones((KC, QB), np.float32))
    assert len(patterns) <= 8, "too many distinct partial-mask patterns"
    return kept, partial, np.stack(patterns)


def _build(kept, partial, nu):
    nc = bacc.Bacc(None, target_bir_lowering=False)
    xT_d = nc.declare_dram_parameter("xT", [D, S], f32, isOutput=False)
    wq_d = nc.declare_dram_parameter("wq", [D, EH], f32, isOutput=False)
    wk_d = nc.declare_dram_parameter("wk", [D, EH], f32, isOutput=False)
    wv_d = nc.declare_dram_parameter("wv", [D, EH], f32, isOutput=False)
    wo_d = nc.declare_dram_parameter("wo", [EH, D], f32, isOutput=False)
    masks_d = nc.declare_dram_parameter("masks", [nu, KC, QB], f32, isOutput=False)
    ones8_d = nc.declare_dram_parameter("ones8", [128, 8], f32, isOutput=False)
    onesr_d = nc.declare_dram_parameter("ones_row", [1, 128], f32, isOutput=False)
    y_d = nc.declare_dram_parameter("y", [S, D], f32, isOutput=True)

    Exp = mybir.ActivationFunctionType.Exp

    with tile.TileContext(nc) as tc, \
         nc.allow_low_precision(reason="fp32r compute tiles share fp32 bits"):
        with tc.tile_pool(name="persist", bufs=1) as persist:
            KT = [persist.tile([128, S], f32r, name=f"kt{t}") for t in range(4)]
            vaug = persist.tile([128, NKC, 8 * VW], f32r, name="vaug")
            aoT = [persist.tile([128, S], f32r, name=f"aot{t}") for t in range(4)]
            onesr = persist.tile([1, 128], f32r, name="onesr")
            maskt = persist.tile([KC, nu, QB], f32r, name="maskt")
            nc.sync.dma_start(out=onesr, in_=onesr_d[:, :].bitcast(f32r))
            for u in range(nu):
                nc.sync.dma_start(out=maskt[:, u, :],
                                  in_=masks_d[u, :, :].bitcast(f32r))

            # ---------------- phase 1: K^T and V projections ----------------
            with tc.tile_pool(name="ph1w", bufs=1) as ph1w, \
                 tc.tile_pool(name="ph1x", bufs=2) as ph1x, \
                 tc.tile_pool(name="ps1", bufs=2, space="PSUM") as ps1:
                wk = ph1w.tile([128, NDC, EH], f32r, name="wk")
                wv = ph1w.tile([128, NDC, EH], f32r, name="wv")
                nc.sync.dma_start(
                    out=wk, in_=wk_d[:, :].rearrange("(dc p) e -> p dc e", p=128).bitcast(f32r))
                nc.sync.dma_start(
                    out=wv, in_=wv_d[:, :].rearrange("(dc p) e -> p dc e", p=128).bitcast(f32r))
                for n in range(NQ):
                    xt = ph1x.tile([128, NDC, QB], f32r, name="xt")
                    nc.sync.dma_start(
                        out=xt,
                        in_=xT_d[:, n * QB:(n + 1) * QB]
                            .rearrange("(dc p) s -> p dc s", p=128).bitcast(f32r))
                    for t in range(4):
                        pk = ps1.tile([128, QB], f32, name="pk")
                        for dc_ in range(NDC):
                            nc.tensor.matmul(
                                pk[:, :], wk[:, dc_, t * 128:(t + 1) * 128],
                                xt[:, dc_, :],
                                start=(dc_ == 0), stop=(dc_ == NDC - 1))
                        nc.vector.tensor_copy(
                            KT[t][:, n * QB:(n + 1) * QB], pk[:, :].bitcast(f32r))
                    for sv in range(4):
                        ci = n * 4 + sv
                        pvv = ps1.tile([128, EH], f32, name="pvv")
                        for dc_ in range(NDC):
                            nc.tensor.matmul(
                                pvv[:, :], xt[:, dc_, sv * 128:(sv + 1) * 128],
                                wv[:, dc_, :],
                                start=(dc_ == 0), stop=(dc_ == NDC - 1))
                        nc.vector.tensor_copy(
                            vaug[:, ci, :].rearrange("p (h w) -> p h w", w=VW)[:, :, 0:DK],
                            pvv[:, :].rearrange("p (h w) -> p h w", w=DK).bitcast(f32r))
                        nc.sync.dma_start(
                            out=vaug[:, ci, :].rearrange("p (h w) -> p h w", w=VW)[:, :, DK:VW],
                            in_=ones8_d[:, :].unsqueeze(-1).bitcast(f32r))

            # ------------- phase 2 (attention) + phase 3 (out proj) -------------
            with tc.tile_pool(name="ph2w", bufs=1) as ph2w, \
                 tc.tile_pool(name="ph2x", bufs=1) as ph2x, \
                 tc.tile_pool(name="ph2s", bufs=2) as ph2s, \
                 tc.tile_pool(name="expp", bufs=4) as expp, \
                 tc.tile_pool(name="ph3o", bufs=3) as ph3o, \
                 tc.tile_pool(name="ps_sc", bufs=2, space="PSUM") as ps_sc, \
                 tc.tile_pool(name="ps_q", bufs=1, space="PSUM") as ps_q, \
                 tc.tile_pool(name="ps_pv", bufs=1, space="PSUM") as ps_pv, \
                 tc.tile_pool(name="ps_bc", bufs=1, space="PSUM") as ps_bc:
                wq = ph2w.tile([128, NDC, EH], f32r, name="wq")
                wo = ph2w.tile([128, 4, 2, QB], f32r, name="wo")
                nc.sync.dma_start(
                    out=wq, in_=wq_d[:, :].rearrange("(dc p) e -> p dc e", p=128).bitcast(f32r))
                nc.sync.dma_start(
                    out=wo, in_=wo_d[:, :].rearrange("(t p) (db u) -> p t db u",
                                                     p=128, u=QB).bitcast(f32r))
                for j in range(NQ):
                    xt2 = ph2x.tile([128, NDC, QB], f32r, name="xt2")
                    nc.sync.dma_start(
                        out=xt2,
                        in_=xT_d[:, j * QB:(j + 1) * QB]
                            .rearrange("(dc p) s -> p dc s", p=128).bitcast(f32r))
                    ks = kept[j]
                    C = len(ks)
                    for t in range(4):
                        pq = ps_q.tile([128, QB], f32, name="pq", tag="pq")
                        for dc_ in range(NDC):
                            nc.tensor.matmul(
                                pq[:, :], wq[:, dc_, t * 128:(t + 1) * 128],
                                xt2[:, dc_, :],
                                start=(dc_ == 0), stop=(dc_ == NDC - 1))
                        qt = ph2s.tile([128, QB], f32r, name="qt")
                        nc.vector.tensor_copy(qt[:, :], pq[:, :].bitcast(f32r))

                        pvA = ps_pv.tile([VW, QB], f32, name="pvA", tag="pvA")
                        pvB = ps_pv.tile([VW, QB], f32, name="pvB", tag="pvB")
                        for idx, c in enumerate(ks):
                            sc = ps_sc.tile([128, 2, QB], f32, name="sc")
                            nc.tensor.matmul(
                                sc[:, 0, :], KT[t][0:64, c * KC:(c + 1) * KC],
                                qt[0:64, :], start=True, stop=True)
                            nc.tensor.matmul(
                                sc[:, 1, :], KT[t][64:128, c * KC:(c + 1) * KC],
                                qt[64:128, :], start=True, stop=True)
                            et = expp.tile([128, 2, QB], f32r, name="et")
                            nc.scalar.activation(et[:, :, :], sc[:, :, :], Exp,
                                                 scale=0.125)
                            u = partial.get((j, c))
                            if u is not None:
                                nc.vector.tensor_mul(
                                    et[:, 0, :], et[:, 0, :], maskt[:, u, :])
                                nc.vector.tensor_mul(
                                    et[:, 1, :], et[:, 1, :], maskt[:, u, :])
                            nc.tensor.matmul(
                                pvA[:, :], vaug[:, c, 2 * VW * t:2 * VW * t + VW],
                                et[:, 0, :],
                                start=(idx == 0), stop=(idx == C - 1))
                            nc.tensor.matmul(
                                pvB[:, :], vaug[:, c, 2 * VW * t + VW:2 * VW * (t + 1)],
                                et[:, 1, :],
                                start=(idx == 0), stop=(idx == C - 1))
                        for pv, hb in ((pvA, 0), (pvB, 64)):
                            recip = ph2s.tile([1, QB], f32r, name="recip")
                            nc.vector.reciprocal(recip[:, :], pv[DK:VW, :])
                            bc = ps_bc.tile([DK, QB], f32, name="bc")
                            nc.tensor.matmul(bc[:, :], onesr[:, 0:DK],
                                             recip[:, :], start=True, stop=True)
                            bcs = ph2s.tile([DK, QB], f32, name="bcs")
                            nc.vector.tensor_copy(bcs[:, :], bc[:, :])
                            nc.vector.tensor_mul(
                                aoT[t][hb:hb + DK, j * QB:(j + 1) * QB],
                                pv[0:DK, :].bitcast(f32r),
                                bcs[:, :].bitcast(f32r))
                    # phase 3 for this q-block's s-chunks
                    for sv in range(4):
                        si = j * 4 + sv
                        for db in range(2):
                            py = ps_q.tile([128, QB], f32, name="py", tag="pq")
                            for t in range(4):
                                nc.tensor.matmul(
                                    py[:, :], aoT[t][:, si * KC:(si + 1) * KC],
                                    wo[:, t, db, :],
                                    start=(t == 0), stop=(t == 3))
                            ys = ph3o.tile([128, QB], f32, name="ys")
                            nc.vector.tensor_copy(ys[:, :], py[:, :])
                            nc.sync.dma_start(
                                out=y_d[si * KC:(si + 1) * KC, db * QB:(db + 1) * QB],
                                in_=ys[:, :])
    nc.finalize()
    return nc


def kernel(x, mask, w_qkv, w_out):
    global LAST_NC, LAST_IN_MAPS
    x = np.ascontiguousarray(np.asarray(x), dtype=np.float32)
    mask = np.asarray(mask)
    w_qkv = np.ascontiguousarray(np.asarray(w_qkv), dtype=np.float32)
    w_out = np.ascontiguousarray(np.asarray(w_out), dtype=np.float32)

    kept, partial, patterns = _classify(mask[0, 0])
    key = (tuple(tuple(k) for k in kept), tuple(sorted(partial.items())),
           len(patterns))
    nc = _cache.get(key)
    if nc is None:
        nc = _build(kept, partial, len(patterns))
        _cache[key] = nc

    ones8 = np.ones((128, 8), np.float32)
    ones_row = np.ones((1, 128), np.float32)
    in_maps = []
    for c in range(8):
        b, g = divmod(c, 2)
        in_maps.append({
            "xT": np.ascontiguousarray(x[b].T),
            "wq": np.ascontiguousarray(w_qkv[:, g * EH:(g + 1) * EH]),
            "wk": np.ascontiguousarray(w_qkv[:, D + g * EH:D + (g + 1) * EH]),
            "wv": np.ascontiguousarray(w_qkv[:, 2 * D + g * EH:2 * D + (g + 1) * EH]),
            "wo": np.ascontiguousarray(w_out[g * EH:(g + 1) * EH, :]),
            "masks": patterns,
            "ones8": ones8,
            "ones_row": ones_row,
        })
    LAST_NC, LAST_IN_MAPS = nc, in_maps

    res = run_bass_kernel_spmd(nc, in_maps, core_ids=list(range(8)))
    y = np.empty((B, S, D), np.float32)
    for b in range(B):
        y[b] = res.results[2 * b]["y"] + res.results[2 * b + 1]["y"]
    return y
